# revision 1
# baseline (speedup 1.0000x reference)
import os

import numpy as np

import concourse.bass as bass
import concourse.mybir as mybir
from concourse.bacc import Bacc
from concourse import bass_utils
from concourse.tile import TileContext

F16 = mybir.dt.float16
F32 = mybir.dt.float32

B, L, D = 16384, 50, 32
NCORES = 8
BC = B // NCORES            # 2048 samples per core
T = BC * L                  # 102400 tokens per core
CHUNK = 512                 # phase-1 token chunk
NCH = T // CHUNK            # 200
HALF = T // 2               # e-strip half (sample aligned: 51200 = 1024*50)
NBLK = 16                   # sample blocks of 128
CPB = 64                    # 100-token chunks per block (64*16*100 = 102400)
MASKV = -60000.0


def _build_program():
    nc = Bacc()
    f16, f32 = F16, F32
    XT = nc.dram_tensor("XT", [128, T], f16, kind="ExternalInput")
    CR = nc.dram_tensor("CR", [65, T], f16, kind="ExternalInput")
    MA = nc.dram_tensor("MA", [1, T], f16, kind="ExternalInput")
    HR = nc.dram_tensor("HR", [128, 1024, 65], f16, kind="ExternalInput")
    UT = nc.dram_tensor("UT", [32, BC], f16, kind="ExternalInput")
    CT = nc.dram_tensor("CT", [64, BC], f16, kind="ExternalInput")
    W1 = nc.dram_tensor("W1", [128, 80], f16, kind="ExternalInput")
    WQ = nc.dram_tensor("WQ", [65, 80], f16, kind="ExternalInput")
    A2R = nc.dram_tensor("A2R", [81, 1], f16, kind="ExternalInput")
    M1 = nc.dram_tensor("M1", [160, 256], f16, kind="ExternalInput")
    MB1 = nc.dram_tensor("MB1", [128, 2], f32, kind="ExternalInput")
    M2 = nc.dram_tensor("M2", [256, 128], f16, kind="ExternalInput")
    MB2 = nc.dram_tensor("MB2", [128, 1], f32, kind="ExternalInput")
    M3 = nc.dram_tensor("M3", [128, 1], f16, kind="ExternalInput")
    MB3 = nc.dram_tensor("MB3", [1, 1], f32, kind="ExternalInput")
    OUT = nc.dram_tensor("out", [1, BC], f32, kind="ExternalOutput")
    EDR = nc.dram_tensor("escr", [2, 512, 2, 50], f16, kind="Internal")

    AF = mybir.ActivationFunctionType

    with TileContext(nc) as tc:
        with (
            tc.tile_pool(name="const", bufs=1) as cp,
            tc.tile_pool(name="xt", bufs=3) as xtp,
            tc.tile_pool(name="cr", bufs=3) as crp,
            tc.tile_pool(name="h", bufs=3) as hp,
            tc.tile_pool(name="hr", bufs=2) as hrp,
            tc.tile_pool(name="persist", bufs=1) as pp,
            tc.tile_pool(name="psA", bufs=2, space="PSUM") as psA,
            tc.tile_pool(name="psB", bufs=2, space="PSUM") as psB,
            tc.tile_pool(name="psC", bufs=2, space="PSUM") as psC,
            tc.tile_pool(name="psD", bufs=2, space="PSUM") as psD,
        ):
            # ---- constants ----
            w1t = cp.tile([128, 80], f16)
            nc.sync.dma_start(out=w1t[:, :], in_=W1[:, :])
            wqt = cp.tile([65, 80], f16)
            nc.sync.dma_start(out=wqt[:, :], in_=WQ[:, :])
            a2rt = cp.tile([81, 1], f16)
            nc.sync.dma_start(out=a2rt[:, :], in_=A2R[:, :])
            m1ut = cp.tile([32, 256], f16, tag="m1u")       # mw1 rows 0:32 (user)
            nc.sync.dma_start(out=m1ut[:, :], in_=M1[0:32, :])
            m1ct = cp.tile([64, 256], f16, tag="m1c")       # rows 32:96 (cand)
            nc.sync.dma_start(out=m1ct[:, :], in_=M1[32:96, :])
            m1a1 = cp.tile([32, 256], f16, tag="m1a1")      # rows 96:128 (att lo)
            nc.sync.dma_start(out=m1a1[:, :], in_=M1[96:128, :])
            m1a2 = cp.tile([32, 256], f16, tag="m1a2")      # rows 128:160 (att hi)
            nc.sync.dma_start(out=m1a2[:, :], in_=M1[128:160, :])
            mb1t = cp.tile([128, 2], f32)
            nc.sync.dma_start(out=mb1t[:, :], in_=MB1[:, :])
            m2t = cp.tile([128, 128], f16, tag="m2a")
            nc.sync.dma_start(out=m2t[:, :], in_=M2[0:128, :])
            m2bt = cp.tile([128, 128], f16, tag="m2b")
            nc.sync.dma_start(out=m2bt[:, :], in_=M2[128:256, :])
            mb2t = cp.tile([128, 1], f32)
            nc.sync.dma_start(out=mb2t[:, :], in_=MB2[:, :])
            m3t = cp.tile([128, 1], f16)
            nc.sync.dma_start(out=m3t[:, :], in_=M3[:, :])
            mb3t = cp.tile([1, 1], f32)
            nc.sync.dma_start(out=mb3t[:, :], in_=MB3[:, :])
            utt = cp.tile([32, BC], f16, tag="ut")
            nc.sync.dma_start(out=utt[:, :], in_=UT[:, :])
            ctt = cp.tile([64, BC], f16, tag="ct")
            nc.sync.dma_start(out=ctt[:, :], in_=CT[:, :])
            ones1 = cp.tile([1, 64], f32)
            nc.vector.memset(ones1[:, :], 1.0)

            estrip = pp.tile([1, HALF], f16, tag="estrip")
            ebig = pp.tile([128, 2 * BC], f16, tag="ebig")
            nc.vector.memset(ebig[:, :], 0.0)
            att_sb = pp.tile([128, 2 * 1024], f16, tag="attsb")  # [0:65] used, att^T+den
            attn = pp.tile([64, BC], f16, tag="attn")
            attb = pp.tile([32, BC], f16, tag="attb")
            rbc_sb = pp.tile([64, BC], f16, tag="rbc")
            z1a = pp.tile([128, BC], f16, tag="z1a")
            z1b = pp.tile([128, BC], f16, tag="z1b")
            z2t = pp.tile([128, BC], f16, tag="z2")
            outs = pp.tile([1, BC], f32, tag="outs")
            rec = pp.tile([1, BC], f32, tag="rec")

            # ---- phase 1: h = relu(X@W + cand@WQ + ab1); e = exp(aw2.h + mask) ----
            for par in range(2):
                for kk in range(NCH // 2):
                    k = par * (NCH // 2) + kk
                    off = k * CHUNK
                    xt = xtp.tile([128, CHUNK], f16)
                    nc.sync.dma_start(out=xt[:, :], in_=XT[:, off:off + CHUNK])
                    cr = crp.tile([65, CHUNK], f16)
                    nc.sync.dma_start(out=cr[:, :], in_=CR[:, off:off + CHUNK])
                    h = hp.tile([81, CHUNK], f16)
                    nc.sync.dma_start(out=h[80:81, :], in_=MA[:, off:off + CHUNK])
                    ps = psA.tile([80, CHUNK], f32)
                    nc.tensor.matmul(ps[:, :], w1t[:, :], xt[:, :], start=True, stop=False)
                    nc.tensor.matmul(ps[:, :], wqt[:, :], cr[:, :], start=False, stop=True)
                    nc.scalar.activation(h[0:80, :], ps[:, :], AF.Relu)
                    ss = psB.tile([1, CHUNK], f32)
                    nc.tensor.matmul(ss[:, :], a2rt[:, :], h[0:81, :], start=True, stop=True)
                    pos = kk * CHUNK
                    nc.scalar.activation(estrip[0:1, pos:pos + CHUNK],
                                         ss[0:1, :], AF.Exp)
                # stage this half's e to DRAM (estrip is reused by next half)
                nc.sync.dma_start(
                    out=EDR[par:par + 1].rearrange("p a b c -> p (a b c)"),
                    in_=estrip[0:1, :])

            # ---- e scatter into block-diag E ----
            # EDR[par, c, j, l] = e(sample par*1024+2c+j, l)
            # -> ebig[j*50 + l, par*1024 + 2c + j]
            for par in range(2):
                for j in range(2):
                    src = EDR[par:par + 1, :, j:j + 1, :].rearrange(
                        "p c j l -> p j l c")
                    dst = ebig[j * 50:(j + 1) * 50,
                               par * 1024:(par + 1) * 1024].rearrange(
                        "p (c w) -> p w c", w=2)[:, j:j + 1, :]
                    nc.sync.dma_start(out=dst, in_=src)

            # ---- phase 2: att^T via per-2-sample E matmuls ----
            for blk in range(NBLK):
                hr = hrp.tile([128, CPB, 65], f16)
                nc.sync.dma_start(out=hr[:, :, :],
                                  in_=HR[:, blk * CPB:(blk + 1) * CPB, :])
                aps = psC.tile([65, 128], f32)
                for i in range(CPB):
                    c = blk * CPB + i
                    nc.tensor.matmul(aps[:, 2 * i:2 * i + 2],
                                     hr[0:100, i, :],
                                     ebig[0:100, 2 * c:2 * c + 2],
                                     start=True, stop=True)
                nc.scalar.activation(att_sb[0:65, blk * 128:(blk + 1) * 128],
                                     aps[:, :], AF.Copy)

            # ---- normalize: att_n = att^T / (den + eps) ----
            nc.vector.tensor_scalar_add(rec[:, :], att_sb[64:65, 0:BC], 1e-20)
            nc.vector.reciprocal(rec[:, :], rec[:, :])
            for q in range(BC // CHUNK):
                off = q * CHUNK
                rb = psD.tile([64, CHUNK], f32, tag="mlp")
                nc.tensor.matmul(rb[:, :], ones1[:, :], rec[:, off:off + CHUNK],
                                 start=True, stop=True)
                nc.scalar.activation(rbc_sb[:, off:off + CHUNK], rb[:, :], AF.Copy)
            nc.vector.tensor_mul(attn[:, :], att_sb[0:64, 0:BC], rbc_sb[:, :])
            nc.vector.tensor_copy(attb[:, :], attn[32:64, :])

            # ---- final MLP ----
            for q in range(BC // CHUNK):
                off = q * CHUNK
                for mh in range(2):
                    zp = psD.tile([128, CHUNK], f32, tag="mlp")
                    mc = mh * 128
                    nc.tensor.matmul(zp[:, :], m1ut[:, mc:mc + 128],
                                     utt[:, off:off + CHUNK], start=True, stop=False)
                    nc.tensor.matmul(zp[:, :], m1ct[:, mc:mc + 128],
                                     ctt[:, off:off + CHUNK], start=False, stop=False)
                    nc.tensor.matmul(zp[:, :], m1a1[:, mc:mc + 128],
                                     attn[0:32, off:off + CHUNK], start=False, stop=False)
                    nc.tensor.matmul(zp[:, :], m1a2[:, mc:mc + 128],
                                     attb[:, off:off + CHUNK], start=False, stop=True)
                    zt = z1a if mh == 0 else z1b
                    nc.scalar.activation(zt[:, off:off + CHUNK], zp[:, :], AF.Relu,
                                         bias=mb1t[:, mh:mh + 1])
                z2p = psD.tile([128, CHUNK], f32, tag="mlp")
                nc.tensor.matmul(z2p[:, :], m2t[:, :], z1a[:, off:off + CHUNK],
                                 start=True, stop=False)
                nc.tensor.matmul(z2p[:, :], m2bt[:, :], z1b[:, off:off + CHUNK],
                                 start=False, stop=True)
                nc.scalar.activation(z2t[:, off:off + CHUNK], z2p[:, :], AF.Relu,
                                     bias=mb2t[:, :])
                z3p = psD.tile([1, CHUNK], f32, tag="mlp")
                nc.tensor.matmul(z3p[:, :], m3t[:, :], z2t[:, off:off + CHUNK],
                                 start=True, stop=True)
                nc.scalar.activation(outs[0:1, off:off + CHUNK], z3p[:, :], AF.Copy)
            nc.vector.tensor_scalar_add(outs[:, :], outs[:, :], mb3t[0:1, 0:1])
            nc.sync.dma_start(out=OUT[:, :], in_=outs[:, :])
    return nc


def kernel(customer_id, candidate_good, candidate_class, history_goods,
           history_classes, user_table, item_table, cat_table,
           aw1, ab1, aw2, ab2, mw1, mb1, mw2, mb2, mw3, mb3):
    f16 = np.float16
    cid = np.asarray(customer_id).astype(np.int64)
    cg = np.asarray(candidate_good).astype(np.int64)
    cc = np.asarray(candidate_class).astype(np.int64)
    hg = np.asarray(history_goods).astype(np.int64)
    hc = np.asarray(history_classes).astype(np.int64)
    ut = np.asarray(user_table, np.float32)
    it = np.asarray(item_table, np.float32)
    ct = np.asarray(cat_table, np.float32)
    aw1 = np.asarray(aw1, np.float32)
    aw2_ = np.asarray(aw2, np.float32)
    A1, A2, A3, A4 = aw1[0:64], aw1[64:128], aw1[128:192], aw1[192:256]
    W1w = np.concatenate([A2 - A3, A4], axis=0)          # [128, 80]
    WQw = A1 + A3                                        # [64, 80]
    WQe = np.concatenate([WQw, np.asarray(ab1, np.float32).reshape(1, 80)], axis=0)
    A2Rw = np.concatenate([aw2_.reshape(80, 1),
                           np.ones((1, 1), np.float32)], axis=0)  # [81,1]
    mw1 = np.asarray(mw1, np.float32)
    mb1v = np.asarray(mb1, np.float32)
    mw2 = np.asarray(mw2, np.float32)
    mb2v = np.asarray(mb2, np.float32)
    mw3 = np.asarray(mw3, np.float32)
    mb3v = np.asarray(mb3, np.float32)
    # reorder mw1 K-rows: reference combined = [user(0:32), cand(32:96), att(96:160)]
    # our K order: u(0:32), cand(32:96), att(96:160)  -> same order
    MB1w = np.stack([mb1v[0:128], mb1v[128:256]], axis=1)  # [128, 2]

    nc = _build_program()
    nc.finalize()
    in_maps = []
    for c in range(NCORES):
        sl = slice(c * BC, (c + 1) * BC)
        g = hg[sl]                       # [BC, 50]
        cl = hc[sl]
        ie = it[g.reshape(-1)]           # [T, 32]
        ce = ct[cl.reshape(-1)]
        ci = it[cg[sl]]                  # [BC, 32]
        cca = ct[cc[sl]]
        cand = np.concatenate([ci, cca], axis=1)          # [BC, 64]
        crep = np.repeat(cand, L, axis=0)                 # [T, 64]
        qhi = ie * crep[:, 0:32]
        qhc = ce * crep[:, 32:64]
        XTa = np.concatenate([ie, ce, qhi, qhc], axis=1).T.astype(f16)  # [128,T]
        CRa = np.concatenate([crep.T, np.ones((1, T), np.float32)],
                             axis=0).astype(f16)
        MAa = np.where(g.reshape(1, -1) == 0, np.float32(MASKV),
                       np.float32(0.0)).astype(f16)
        hrow = np.concatenate([ie, ce, np.ones((T, 1), np.float32)],
                              axis=1)                     # [T, 65]
        HRa = np.zeros((128, 1024, 65), f16)
        HRa[0:100, :, :] = hrow.reshape(1024, 100, 65).transpose(1, 0, 2).astype(f16)
        in_maps.append(dict(
            XT=XTa, CR=CRa, MA=MAa, HR=HRa,
            UT=ut[cid[sl]].T.astype(f16), CT=cand.T.astype(f16),
            W1=W1w.astype(f16), WQ=WQe.astype(f16), A2R=A2Rw.astype(f16),
            M1=mw1.astype(f16), MB1=MB1w,
            M2=mw2.astype(f16), MB2=mb2v.reshape(128, 1),
            M3=mw3.astype(f16), MB3=mb3v.reshape(1, 1),
            ))
    import time as _time
    _t0 = _time.time()
    res = bass_utils.run_bass_kernel_spmd(
        nc, in_maps, core_ids=list(range(NCORES)))
    _t1 = _time.time()
    if res.exec_time_ns:
        print(f"HW exec time: {res.exec_time_ns} ns")
    else:
        print(f"HW exec time: {int((_t1 - _t0) * 1e9)} ns (execute-call wall; "
              f"NTFF profiling unavailable under this axon client)")
    outs = [np.asarray(r["out"]).reshape(-1) for r in res.results]
    return np.concatenate(outs).astype(np.float32)



# revision 9
# speedup vs baseline: 3.3458x; 3.3458x over previous
import time

import numpy as np

import concourse.bass as bass
import concourse.mybir as mybir
from concourse.bacc import Bacc
from concourse.tile import TileContext

F16 = mybir.dt.float16
F32 = mybir.dt.float32

B, L, D = 16384, 50, 32
NCORES = 8
BC = B // NCORES            # 2048 samples per core
SCH = 8                     # samples per loop chunk
TCH = SCH * L               # 400 tokens per chunk
MASKV = -60000.0


def _build_program():
    nc = Bacc()
    f16, f32 = F16, F32
    AF = mybir.ActivationFunctionType
    ds = bass.ds

    HT = nc.dram_tensor("HT", [65, BC, L], f16, kind="ExternalInput")
    CA = nc.dram_tensor("CA", [64, BC], f16, kind="ExternalInput")
    UT = nc.dram_tensor("UT", [32, BC], f16, kind="ExternalInput")
    W1H = nc.dram_tensor("W1H", [64, 80], f16, kind="ExternalInput")
    W1P = nc.dram_tensor("W1P", [64, 80], f16, kind="ExternalInput")
    WQC = nc.dram_tensor("WQC", [64, 80], f16, kind="ExternalInput")
    AB1 = nc.dram_tensor("AB1", [80, 1], f32, kind="ExternalInput")
    A2E = nc.dram_tensor("A2E", [80, 1], f16, kind="ExternalInput")
    M1U = nc.dram_tensor("M1U", [32, 256], f16, kind="ExternalInput")
    M1C = nc.dram_tensor("M1C", [64, 256], f16, kind="ExternalInput")
    M1A = nc.dram_tensor("M1A", [64, 256], f16, kind="ExternalInput")
    MB1 = nc.dram_tensor("MB1", [128, 2], f32, kind="ExternalInput")
    M2A = nc.dram_tensor("M2A", [128, 128], f16, kind="ExternalInput")
    M2B = nc.dram_tensor("M2B", [128, 128], f16, kind="ExternalInput")
    MB2 = nc.dram_tensor("MB2", [128, 1], f32, kind="ExternalInput")
    M3 = nc.dram_tensor("M3", [128, 1], f16, kind="ExternalInput")
    MB3 = nc.dram_tensor("MB3", [1, 1], f32, kind="ExternalInput")
    OUT = nc.dram_tensor("out", [1, BC], f32, kind="ExternalOutput")

    with TileContext(nc) as tc:
        with (
            tc.tile_pool(name="const", bufs=1) as cp,
            tc.tile_pool(name="ht", bufs=2) as htp,
            tc.tile_pool(name="work", bufs=2) as wp,
            tc.tile_pool(name="mlp", bufs=2) as mp,
            tc.tile_pool(name="psA", bufs=1, space="PSUM") as psA,
            tc.tile_pool(name="psS", bufs=1, space="PSUM") as psS,
            tc.tile_pool(name="psE", bufs=1, space="PSUM") as psE,
            tc.tile_pool(name="psM", bufs=2, space="PSUM") as psM,
        ):
            # ---- constants ----
            w1h = cp.tile([64, 80], f16, tag="w1h")
            nc.sync.dma_start(out=w1h[:, :], in_=W1H[:, :])
            w1p = cp.tile([64, 80], f16, tag="w1p")
            nc.sync.dma_start(out=w1p[:, :], in_=W1P[:, :])
            wqc = cp.tile([64, 80], f16, tag="wqc")
            nc.sync.dma_start(out=wqc[:, :], in_=WQC[:, :])
            ab1 = cp.tile([80, 1], f32, tag="ab1")
            nc.sync.dma_start(out=ab1[:, :], in_=AB1[:, :])
            a2e = cp.tile([80, 1], f16, tag="a2e")
            nc.sync.dma_start(out=a2e[:, :], in_=A2E[:, :])
            m1u = cp.tile([32, 256], f16, tag="m1u")
            nc.sync.dma_start(out=m1u[:, :], in_=M1U[:, :])
            m1c = cp.tile([64, 256], f16, tag="m1c")
            nc.sync.dma_start(out=m1c[:, :], in_=M1C[:, :])
            m1a = cp.tile([64, 256], f16, tag="m1a")
            nc.sync.dma_start(out=m1a[:, :], in_=M1A[:, :])
            mb1 = cp.tile([128, 2], f32, tag="mb1")
            nc.sync.dma_start(out=mb1[:, :], in_=MB1[:, :])
            m2a = cp.tile([128, 128], f16, tag="m2a")
            nc.sync.dma_start(out=m2a[:, :], in_=M2A[:, :])
            m2b = cp.tile([128, 128], f16, tag="m2b")
            nc.sync.dma_start(out=m2b[:, :], in_=M2B[:, :])
            mb2 = cp.tile([128, 1], f32, tag="mb2")
            nc.sync.dma_start(out=mb2[:, :], in_=MB2[:, :])
            m3 = cp.tile([128, 1], f16, tag="m3")
            nc.sync.dma_start(out=m3[:, :], in_=M3[:, :])
            mb3 = cp.tile([1, 1], f32, tag="mb3")
            nc.sync.dma_start(out=mb3[:, :], in_=MB3[:, :])
            ca = cp.tile([64, BC], f16, tag="ca")
            nc.sync.dma_start(out=ca[:, :], in_=CA[:, :])
            ut = cp.tile([32, BC], f16, tag="ut")
            nc.sync.dma_start(out=ut[:, :], in_=UT[:, :])
            ones1 = cp.tile([1, 64], f16, tag="ones1")
            nc.vector.memset(ones1[:, :], 1.0)
            onesm = cp.tile([65, 1], f16, tag="onesm")
            nc.vector.memset(onesm[:, :], 1.0)

            attS = cp.tile([64, BC], f32, tag="attS")
            denS = cp.tile([1, BC], f32, tag="denS")
            attn = cp.tile([64, BC], f16, tag="attn")
            rec = cp.tile([1, BC], f32, tag="rec")
            rech = cp.tile([1, BC], f16, tag="rech")
            z1a = cp.tile([128, BC], f16, tag="z1a")
            z1b = cp.tile([128, BC], f16, tag="z1b")
            z2t = cp.tile([128, BC], f16, tag="z2")
            outs = cp.tile([1, BC], f32, tag="outs")

            # ---- fused attention loop: 8 samples (400 tokens) per iter ----
            with tc.For_i(0, BC, SCH) as i:
                ht = htp.tile([65, TCH], f16)
                nc.sync.dma_start(
                    out=ht[:, :].rearrange("p (s l) -> p s l", l=L),
                    in_=HT[:, ds(i, SCH), :])
                prod = wp.tile([64, TCH], f16)
                nc.vector.tensor_tensor(
                    out=prod[:, :].rearrange("p (s l) -> p s l", l=L),
                    in0=ht[0:64, :].rearrange("p (s l) -> p s l", l=L),
                    in1=ca[:, ds(i, SCH)].to_broadcast([64, SCH, L]),
                    op=mybir.AluOpType.mult)
                ps = psA.tile([80, TCH], f32)
                nc.tensor.matmul(ps[:, :], w1h[:, :], ht[0:64, :],
                                 start=True, stop=False)
                nc.tensor.matmul(ps[:, :], w1p[:, :], prod[:, :],
                                 start=False, stop=False)
                nc.tensor.matmul(ps[:, :],
                                 wqc[:, :],
                                 ca[:, ds(i, SCH)].to_broadcast([64, SCH, L]),
                                 start=False, stop=True)
                h80 = wp.tile([80, TCH], f16)
                nc.scalar.activation(h80[:, :], ps[:, :], AF.Relu,
                                     bias=ab1[:, :])
                ss = psS.tile([1, TCH], f32)
                nc.tensor.matmul(ss[:, :], a2e[:, :], h80[:, :],
                                 start=True, stop=False)
                nc.tensor.matmul(ss[:, :], onesm[64:65, :], ht[64:65, :],
                                 start=False, stop=True)
                e = wp.tile([1, TCH], f16)
                nc.scalar.activation(e[:, :], ss[:, :], AF.Exp)
                ebc = psE.tile([64, TCH], f32)
                nc.tensor.matmul(ebc[:, :], ones1[:, :], e[:, :],
                                 start=True, stop=True)
                wh = wp.tile([64, TCH], f32)
                nc.vector.tensor_tensor(out=wh[:, :], in0=ht[0:64, :],
                                        in1=ebc[:, :],
                                        op=mybir.AluOpType.mult)
                nc.vector.tensor_reduce(
                    out=attS[:, ds(i, SCH)],
                    in_=wh[:, :].rearrange("p (s l) -> p s l", l=L),
                    axis=mybir.AxisListType.X, op=mybir.AluOpType.add)
                nc.vector.tensor_reduce(
                    out=denS[:, ds(i, SCH)],
                    in_=e[:, :].rearrange("p (s l) -> p s l", l=L),
                    axis=mybir.AxisListType.X, op=mybir.AluOpType.add)

            # ---- normalize attention ----
            nc.vector.tensor_scalar_add(rec[:, :], denS[:, :], 1e-20)
            nc.vector.reciprocal(rec[:, :], rec[:, :])
            nc.scalar.activation(rech[:, :], rec[:, :], AF.Copy)
            CH = 512
            for q in range(BC // CH):
                off = q * CH
                rb = psM.tile([64, CH], f32, tag="m")
                nc.tensor.matmul(rb[:, :], ones1[:, :], rech[:, off:off + CH],
                                 start=True, stop=True)
                nc.vector.tensor_tensor(out=attn[:, off:off + CH],
                                        in0=attS[:, off:off + CH],
                                        in1=rb[:, :],
                                        op=mybir.AluOpType.mult)

            # ---- final MLP ----
            for q in range(BC // CH):
                off = q * CH
                sl = slice(off, off + CH)
                for mh in range(2):
                    mc = mh * 128
                    zp = psM.tile([128, CH], f32, tag="m")
                    nc.tensor.matmul(zp[:, :], m1u[:, mc:mc + 128],
                                     ut[:, sl], start=True, stop=False)
                    nc.tensor.matmul(zp[:, :], m1c[:, mc:mc + 128],
                                     ca[:, sl], start=False, stop=False)
                    nc.tensor.matmul(zp[:, :], m1a[:, mc:mc + 128],
                                     attn[:, sl], start=False, stop=True)
                    zt = z1a if mh == 0 else z1b
                    nc.scalar.activation(zt[:, sl], zp[:, :], AF.Relu,
                                         bias=mb1[:, mh:mh + 1])
                z2p = psM.tile([128, CH], f32, tag="m")
                nc.tensor.matmul(z2p[:, :], m2a[:, :], z1a[:, sl],
                                 start=True, stop=False)
                nc.tensor.matmul(z2p[:, :], m2b[:, :], z1b[:, sl],
                                 start=False, stop=True)
                nc.scalar.activation(z2t[:, sl], z2p[:, :], AF.Relu,
                                     bias=mb2[:, :])
                z3p = psM.tile([1, CH], f32, tag="m")
                nc.tensor.matmul(z3p[:, :], m3[:, :], z2t[:, sl],
                                 start=True, stop=True)
                nc.scalar.activation(outs[:, off:off + CH], z3p[:, :], AF.Copy)
            nc.vector.tensor_scalar_add(outs[:, :], outs[:, :], mb3[0:1, 0:1])
            nc.sync.dma_start(out=OUT[:, :], in_=outs[:, :])
    return nc


def _run(nc, global_ins, n_cores):
    """Execute the finalized program on n_cores via PJRT (axon).

    Inputs are shipped with per-device device_put (async) so the transfer
    overlaps the NEFF compile/load; the jit call then consumes resident
    arrays. Returns (outputs, exec_ns) where exec_ns covers device_put
    issue + compile + execute + fetch.
    """
    import jax
    from jax.sharding import Mesh, PartitionSpec, NamedSharding
    try:
        from jax import shard_map
        def _smap(f, mesh, in_specs, out_specs):
            return shard_map(f, mesh=mesh, in_specs=in_specs,
                             out_specs=out_specs, check_vma=False)
    except ImportError:
        from jax.experimental.shard_map import shard_map
        def _smap(f, mesh, in_specs, out_specs):
            return shard_map(f, mesh=mesh, in_specs=in_specs,
                             out_specs=out_specs, check_rep=False)
    from concourse import bass2jax

    devs = jax.devices()[:n_cores]
    bass2jax.install_neuronx_cc_hook()

    partition_name = (nc.partition_id_tensor.name
                      if nc.partition_id_tensor else None)
    in_names, out_names, out_avals, zero_outs = [], [], [], []
    for alloc in nc.m.functions[0].allocations:
        if not isinstance(alloc, mybir.MemoryLocationSet):
            continue
        name = alloc.memorylocations[0].name
        if alloc.kind == "ExternalInput":
            if name != partition_name:
                in_names.append(name)
        elif alloc.kind == "ExternalOutput":
            shape = tuple(alloc.tensor_shape)
            dtype = mybir.dt.np(alloc.dtype)
            out_avals.append(jax.core.ShapedArray(shape, dtype))
            out_names.append(name)
            zero_outs.append(np.zeros((n_cores * shape[0], *shape[1:]), dtype))
    n_params = len(in_names)
    n_outs = len(out_avals)
    all_names = list(in_names) + list(out_names)
    if partition_name is not None:
        all_names.append(partition_name)

    def _body(*args):
        operands = list(args)
        if partition_name is not None:
            operands.append(bass2jax.partition_id_tensor())
        return tuple(bass2jax._bass_exec_p.bind(
            *operands, out_avals=tuple(out_avals), in_names=tuple(all_names),
            out_names=tuple(out_names), lowering_input_output_aliases=(),
            sim_require_finite=True, sim_require_nnan=True, nc=nc))

    donate = tuple(range(n_params, n_params + n_outs))
    mesh = Mesh(np.asarray(devs), ("core",))
    sharded = jax.jit(
        _smap(_body, mesh,
              (PartitionSpec("core"),) * (n_params + n_outs),
              (PartitionSpec("core"),) * n_outs),
        donate_argnums=donate, keep_unused=True)

    t0 = time.time()
    # async per-device puts: transfer streams while the NEFF compiles below
    sh = NamedSharding(mesh, PartitionSpec("core"))
    dev_in = []
    for name in in_names:
        a = global_ins[name]
        per = a.shape[0] // n_cores
        shards = [jax.device_put(a[c * per:(c + 1) * per], devs[c])
                  for c in range(n_cores)]
        dev_in.append(jax.make_array_from_single_device_arrays(
            a.shape, sh, shards))
    dev_zero = []
    for z in zero_outs:
        per = z.shape[0] // n_cores
        shards = [jax.device_put(z[c * per:(c + 1) * per], devs[c])
                  for c in range(n_cores)]
        dev_zero.append(jax.make_array_from_single_device_arrays(
            z.shape, sh, shards))

    compiled = sharded.lower(*dev_in, *dev_zero).compile()
    out_arrs = compiled(*dev_in, *dev_zero)
    res = [np.asarray(o) for o in out_arrs]
    t1 = time.time()
    print(f"HW exec time: {int((t1 - t0) * 1e9)} ns")
    return {name: res[k] for k, name in enumerate(out_names)}


def kernel(customer_id, candidate_good, candidate_class, history_goods,
           history_classes, user_table, item_table, cat_table,
           aw1, ab1, aw2, ab2, mw1, mb1, mw2, mb2, mw3, mb3):
    f16 = np.float16
    cid = np.asarray(customer_id).astype(np.int64)
    cg = np.asarray(candidate_good).astype(np.int64)
    cc = np.asarray(candidate_class).astype(np.int64)
    hg = np.asarray(history_goods).astype(np.int64)
    hc = np.asarray(history_classes).astype(np.int64)
    ut = np.asarray(user_table, np.float32)
    it = np.asarray(item_table, np.float32)
    ct = np.asarray(cat_table, np.float32)
    aw1 = np.asarray(aw1, np.float32)
    aw2_ = np.asarray(aw2, np.float32).reshape(80, 1)
    A1, A2, A3, A4 = aw1[0:64], aw1[64:128], aw1[128:192], aw1[192:256]
    mw1 = np.asarray(mw1, np.float32)
    mb1v = np.asarray(mb1, np.float32)
    mw2 = np.asarray(mw2, np.float32)
    mw3 = np.asarray(mw3, np.float32)

    nc = _build_program()
    nc.finalize()

    # ---- host-side gather into compact device layouts (f16) ----
    ieT = it[hg].transpose(2, 0, 1).astype(f16)      # [32, B, 50]
    ceT = ct[hc].transpose(2, 0, 1).astype(f16)      # [32, B, 50]
    maT = np.where(hg == 0, np.float32(MASKV),
                   np.float32(0.0)).astype(f16)[None]  # [1, B, 50]
    HTg = np.concatenate([ieT, ceT, maT], axis=0)    # [65, B, 50]
    HTg = np.concatenate(
        [HTg[:, c * BC:(c + 1) * BC] for c in range(NCORES)], axis=0)
    CAg = np.concatenate([it[cg].T, ct[cc].T], axis=0).astype(f16)  # [64, B]
    CAg = np.concatenate(
        [CAg[:, c * BC:(c + 1) * BC] for c in range(NCORES)], axis=0)
    UTg = ut[cid].T.astype(f16)                       # [32, B]
    UTg = np.concatenate(
        [UTg[:, c * BC:(c + 1) * BC] for c in range(NCORES)], axis=0)

    W1H = np.ascontiguousarray(A2 - A3).astype(f16)
    W1P = np.ascontiguousarray(A4).astype(f16)
    WQC = np.ascontiguousarray(A1 + A3).astype(f16)
    AB1 = np.asarray(ab1, np.float32).reshape(80, 1)
    MB1w = np.stack([mb1v[0:128], mb1v[128:256]], axis=1)

    def rep(a):
        return np.concatenate([a] * NCORES, axis=0)

    global_ins = dict(
        HT=HTg, CA=CAg, UT=UTg,
        W1H=rep(W1H), W1P=rep(W1P), WQC=rep(WQC), AB1=rep(AB1),
        A2E=rep(aw2_.astype(f16)),
        M1U=rep(mw1[0:32].astype(f16)), M1C=rep(mw1[32:96].astype(f16)),
        M1A=rep(mw1[96:160].astype(f16)), MB1=rep(MB1w),
        M2A=rep(mw2[0:128].astype(f16)), M2B=rep(mw2[128:256].astype(f16)),
        MB2=rep(np.asarray(mb2, np.float32).reshape(128, 1)),
        M3=rep(mw3.astype(f16)),
        MB3=rep(np.asarray(mb3, np.float32).reshape(1, 1)),
    )
    res = _run(nc, global_ins, NCORES)
    return res["out"].reshape(-1).astype(np.float32)


# revision 10
# speedup vs baseline: 3.4437x; 1.0293x over previous
import time

import numpy as np

import concourse.bass as bass
import concourse.mybir as mybir
from concourse.bacc import Bacc
from concourse.tile import TileContext

F16 = mybir.dt.float16
F32 = mybir.dt.float32

B, L, D = 16384, 50, 32
NCORES = 8
BC = B // NCORES            # 2048 samples per core
SCH = 8                     # samples per loop chunk
TCH = SCH * L               # 400 tokens per chunk
MASKV = -60000.0


def _build_program():
    nc = Bacc()
    f16, f32 = F16, F32
    AF = mybir.ActivationFunctionType
    ds = bass.ds

    HT = nc.dram_tensor("HT", [65, BC, L], f16, kind="ExternalInput")
    CA = nc.dram_tensor("CA", [64, BC], f16, kind="ExternalInput")
    UT = nc.dram_tensor("UT", [32, BC], f16, kind="ExternalInput")
    W1H = nc.dram_tensor("W1H", [64, 80], f16, kind="ExternalInput")
    W1P = nc.dram_tensor("W1P", [64, 80], f16, kind="ExternalInput")
    WQC = nc.dram_tensor("WQC", [64, 80], f16, kind="ExternalInput")
    AB1 = nc.dram_tensor("AB1", [80, 1], f32, kind="ExternalInput")
    A2E = nc.dram_tensor("A2E", [80, 1], f16, kind="ExternalInput")
    M1U = nc.dram_tensor("M1U", [32, 256], f16, kind="ExternalInput")
    M1C = nc.dram_tensor("M1C", [64, 256], f16, kind="ExternalInput")
    M1A = nc.dram_tensor("M1A", [64, 256], f16, kind="ExternalInput")
    MB1 = nc.dram_tensor("MB1", [128, 2], f32, kind="ExternalInput")
    M2A = nc.dram_tensor("M2A", [128, 128], f16, kind="ExternalInput")
    M2B = nc.dram_tensor("M2B", [128, 128], f16, kind="ExternalInput")
    MB2 = nc.dram_tensor("MB2", [128, 1], f32, kind="ExternalInput")
    M3 = nc.dram_tensor("M3", [128, 1], f16, kind="ExternalInput")
    MB3 = nc.dram_tensor("MB3", [1, 1], f32, kind="ExternalInput")
    OUT = nc.dram_tensor("out", [1, BC], f32, kind="ExternalOutput")

    with TileContext(nc) as tc:
        with (
            tc.tile_pool(name="const", bufs=1) as cp,
            tc.tile_pool(name="ht", bufs=2) as htp,
            tc.tile_pool(name="work", bufs=2) as wp,
            tc.tile_pool(name="mlp", bufs=2) as mp,
            tc.tile_pool(name="psA", bufs=1, space="PSUM") as psA,
            tc.tile_pool(name="psS", bufs=1, space="PSUM") as psS,
            tc.tile_pool(name="psE", bufs=1, space="PSUM") as psE,
            tc.tile_pool(name="psM", bufs=2, space="PSUM") as psM,
        ):
            # ---- constants ----
            w1h = cp.tile([64, 80], f16, tag="w1h")
            nc.sync.dma_start(out=w1h[:, :], in_=W1H[:, :])
            w1p = cp.tile([64, 80], f16, tag="w1p")
            nc.sync.dma_start(out=w1p[:, :], in_=W1P[:, :])
            wqc = cp.tile([64, 80], f16, tag="wqc")
            nc.sync.dma_start(out=wqc[:, :], in_=WQC[:, :])
            ab1 = cp.tile([80, 1], f32, tag="ab1")
            nc.sync.dma_start(out=ab1[:, :], in_=AB1[:, :])
            a2e = cp.tile([80, 1], f16, tag="a2e")
            nc.sync.dma_start(out=a2e[:, :], in_=A2E[:, :])
            m1u = cp.tile([32, 256], f16, tag="m1u")
            nc.sync.dma_start(out=m1u[:, :], in_=M1U[:, :])
            m1c = cp.tile([64, 256], f16, tag="m1c")
            nc.sync.dma_start(out=m1c[:, :], in_=M1C[:, :])
            m1a = cp.tile([64, 256], f16, tag="m1a")
            nc.sync.dma_start(out=m1a[:, :], in_=M1A[:, :])
            mb1 = cp.tile([128, 2], f32, tag="mb1")
            nc.sync.dma_start(out=mb1[:, :], in_=MB1[:, :])
            m2a = cp.tile([128, 128], f16, tag="m2a")
            nc.sync.dma_start(out=m2a[:, :], in_=M2A[:, :])
            m2b = cp.tile([128, 128], f16, tag="m2b")
            nc.sync.dma_start(out=m2b[:, :], in_=M2B[:, :])
            mb2 = cp.tile([128, 1], f32, tag="mb2")
            nc.sync.dma_start(out=mb2[:, :], in_=MB2[:, :])
            m3 = cp.tile([128, 1], f16, tag="m3")
            nc.sync.dma_start(out=m3[:, :], in_=M3[:, :])
            mb3 = cp.tile([1, 1], f32, tag="mb3")
            nc.sync.dma_start(out=mb3[:, :], in_=MB3[:, :])
            ca = cp.tile([64, BC], f16, tag="ca")
            nc.sync.dma_start(out=ca[:, :], in_=CA[:, :])
            ut = cp.tile([32, BC], f16, tag="ut")
            nc.sync.dma_start(out=ut[:, :], in_=UT[:, :])
            ones1 = cp.tile([1, 64], f16, tag="ones1")
            nc.vector.memset(ones1[:, :], 1.0)
            onesm = cp.tile([65, 1], f16, tag="onesm")
            nc.vector.memset(onesm[:, :], 1.0)

            attS = cp.tile([64, BC], f32, tag="attS")
            denS = cp.tile([1, BC], f32, tag="denS")
            attn = cp.tile([64, BC], f16, tag="attn")
            rec = cp.tile([1, BC], f32, tag="rec")
            rech = cp.tile([1, BC], f16, tag="rech")
            z1a = cp.tile([128, BC], f16, tag="z1a")
            z1b = cp.tile([128, BC], f16, tag="z1b")
            z2t = cp.tile([128, BC], f16, tag="z2")
            outs = cp.tile([1, BC], f32, tag="outs")

            # ---- fused attention loop: 8 samples (400 tokens) per iter ----
            with tc.For_i(0, BC, SCH) as i:
                ht = htp.tile([65, TCH], f16)
                nc.sync.dma_start(
                    out=ht[:, :].rearrange("p (s l) -> p s l", l=L),
                    in_=HT[:, ds(i, SCH), :])
                prod = wp.tile([64, TCH], f16)
                nc.vector.tensor_tensor(
                    out=prod[:, :].rearrange("p (s l) -> p s l", l=L),
                    in0=ht[0:64, :].rearrange("p (s l) -> p s l", l=L),
                    in1=ca[:, ds(i, SCH)].to_broadcast([64, SCH, L]),
                    op=mybir.AluOpType.mult)
                ps = psA.tile([80, TCH], f32)
                nc.tensor.matmul(ps[:, :], w1h[:, :], ht[0:64, :],
                                 start=True, stop=False)
                nc.tensor.matmul(ps[:, :], w1p[:, :], prod[:, :],
                                 start=False, stop=False)
                nc.tensor.matmul(ps[:, :],
                                 wqc[:, :],
                                 ca[:, ds(i, SCH)].to_broadcast([64, SCH, L]),
                                 start=False, stop=True)
                h80 = wp.tile([80, TCH], f16)
                nc.scalar.activation(h80[:, :], ps[:, :], AF.Relu,
                                     bias=ab1[:, :])
                ss = psS.tile([1, TCH], f32)
                nc.tensor.matmul(ss[:, :], a2e[:, :], h80[:, :],
                                 start=True, stop=False)
                nc.tensor.matmul(ss[:, :], onesm[64:65, :], ht[64:65, :],
                                 start=False, stop=True)
                e = wp.tile([1, TCH], f16)
                nc.scalar.activation(e[:, :], ss[:, :], AF.Exp)
                ebc = psE.tile([64, TCH], f32)
                nc.tensor.matmul(ebc[:, :], ones1[:, :], e[:, :],
                                 start=True, stop=True)
                wh = wp.tile([64, TCH], f32)
                nc.vector.tensor_tensor(out=wh[:, :], in0=ht[0:64, :],
                                        in1=ebc[:, :],
                                        op=mybir.AluOpType.mult)
                nc.vector.tensor_reduce(
                    out=attS[:, ds(i, SCH)],
                    in_=wh[:, :].rearrange("p (s l) -> p s l", l=L),
                    axis=mybir.AxisListType.X, op=mybir.AluOpType.add)
                nc.vector.tensor_reduce(
                    out=denS[:, ds(i, SCH)],
                    in_=e[:, :].rearrange("p (s l) -> p s l", l=L),
                    axis=mybir.AxisListType.X, op=mybir.AluOpType.add)

            # ---- normalize attention ----
            nc.vector.tensor_scalar_add(rec[:, :], denS[:, :], 1e-20)
            nc.vector.reciprocal(rec[:, :], rec[:, :])
            nc.scalar.activation(rech[:, :], rec[:, :], AF.Copy)
            CH = 512
            for q in range(BC // CH):
                off = q * CH
                rb = psM.tile([64, CH], f32, tag="m")
                nc.tensor.matmul(rb[:, :], ones1[:, :], rech[:, off:off + CH],
                                 start=True, stop=True)
                nc.vector.tensor_tensor(out=attn[:, off:off + CH],
                                        in0=attS[:, off:off + CH],
                                        in1=rb[:, :],
                                        op=mybir.AluOpType.mult)

            # ---- final MLP ----
            for q in range(BC // CH):
                off = q * CH
                sl = slice(off, off + CH)
                for mh in range(2):
                    mc = mh * 128
                    zp = psM.tile([128, CH], f32, tag="m")
                    nc.tensor.matmul(zp[:, :], m1u[:, mc:mc + 128],
                                     ut[:, sl], start=True, stop=False)
                    nc.tensor.matmul(zp[:, :], m1c[:, mc:mc + 128],
                                     ca[:, sl], start=False, stop=False)
                    nc.tensor.matmul(zp[:, :], m1a[:, mc:mc + 128],
                                     attn[:, sl], start=False, stop=True)
                    zt = z1a if mh == 0 else z1b
                    nc.scalar.activation(zt[:, sl], zp[:, :], AF.Relu,
                                         bias=mb1[:, mh:mh + 1])
                z2p = psM.tile([128, CH], f32, tag="m")
                nc.tensor.matmul(z2p[:, :], m2a[:, :], z1a[:, sl],
                                 start=True, stop=False)
                nc.tensor.matmul(z2p[:, :], m2b[:, :], z1b[:, sl],
                                 start=False, stop=True)
                nc.scalar.activation(z2t[:, sl], z2p[:, :], AF.Relu,
                                     bias=mb2[:, :])
                z3p = psM.tile([1, CH], f32, tag="m")
                nc.tensor.matmul(z3p[:, :], m3[:, :], z2t[:, sl],
                                 start=True, stop=True)
                nc.scalar.activation(outs[:, off:off + CH], z3p[:, :], AF.Copy)
            nc.vector.tensor_scalar_add(outs[:, :], outs[:, :], mb3[0:1, 0:1])
            nc.sync.dma_start(out=OUT[:, :], in_=outs[:, :])
    return nc


def _run(nc, global_ins, n_cores):
    """Execute the finalized program on n_cores via PJRT (axon).

    Inputs are shipped with per-device device_put (async) so the transfer
    overlaps the NEFF compile/load; the jit call then consumes resident
    arrays. Returns (outputs, exec_ns) where exec_ns covers device_put
    issue + compile + execute + fetch.
    """
    import jax
    from jax.sharding import Mesh, PartitionSpec, NamedSharding
    try:
        from jax import shard_map
        def _smap(f, mesh, in_specs, out_specs):
            return shard_map(f, mesh=mesh, in_specs=in_specs,
                             out_specs=out_specs, check_vma=False)
    except ImportError:
        from jax.experimental.shard_map import shard_map
        def _smap(f, mesh, in_specs, out_specs):
            return shard_map(f, mesh=mesh, in_specs=in_specs,
                             out_specs=out_specs, check_rep=False)
    from concourse import bass2jax

    devs = jax.devices()[:n_cores]
    bass2jax.install_neuronx_cc_hook()

    partition_name = (nc.partition_id_tensor.name
                      if nc.partition_id_tensor else None)
    in_names, out_names, out_avals, zero_outs = [], [], [], []
    for alloc in nc.m.functions[0].allocations:
        if not isinstance(alloc, mybir.MemoryLocationSet):
            continue
        name = alloc.memorylocations[0].name
        if alloc.kind == "ExternalInput":
            if name != partition_name:
                in_names.append(name)
        elif alloc.kind == "ExternalOutput":
            shape = tuple(alloc.tensor_shape)
            dtype = mybir.dt.np(alloc.dtype)
            out_avals.append(jax.core.ShapedArray(shape, dtype))
            out_names.append(name)
            zero_outs.append(np.zeros((n_cores * shape[0], *shape[1:]), dtype))
    n_params = len(in_names)
    n_outs = len(out_avals)
    all_names = list(in_names) + list(out_names)
    if partition_name is not None:
        all_names.append(partition_name)

    def _body(*args):
        operands = list(args)
        if partition_name is not None:
            operands.append(bass2jax.partition_id_tensor())
        return tuple(bass2jax._bass_exec_p.bind(
            *operands, out_avals=tuple(out_avals), in_names=tuple(all_names),
            out_names=tuple(out_names), lowering_input_output_aliases=(),
            sim_require_finite=True, sim_require_nnan=True, nc=nc))

    donate = tuple(range(n_params, n_params + n_outs))
    mesh = Mesh(np.asarray(devs), ("core",))
    sharded = jax.jit(
        _smap(_body, mesh,
              (PartitionSpec("core"),) * (n_params + n_outs),
              (PartitionSpec("core"),) * n_outs),
        donate_argnums=donate, keep_unused=True)

    t0 = time.time()
    # async per-device puts: transfer streams while the NEFF compiles below
    sh = NamedSharding(mesh, PartitionSpec("core"))
    dev_in = []
    for name in in_names:
        a = global_ins[name]
        per = a.shape[0] // n_cores
        shards = [jax.device_put(a[c * per:(c + 1) * per], devs[c])
                  for c in range(n_cores)]
        dev_in.append(jax.make_array_from_single_device_arrays(
            a.shape, sh, shards))
    dev_zero = []
    for z in zero_outs:
        per = z.shape[0] // n_cores
        shards = [jax.device_put(z[c * per:(c + 1) * per], devs[c])
                  for c in range(n_cores)]
        dev_zero.append(jax.make_array_from_single_device_arrays(
            z.shape, sh, shards))

    tp = time.time()
    compiled = sharded.lower(*dev_in, *dev_zero).compile()
    tc_ = time.time()
    out_arrs = compiled(*dev_in, *dev_zero)
    res = [np.asarray(o) for o in out_arrs]
    t1 = time.time()
    print(f"[breakdown] put-issue {tp - t0:.2f}s  compile {tc_ - tp:.2f}s  "
          f"exec+fetch {t1 - tc_:.2f}s")
    print(f"HW exec time: {int((t1 - t0) * 1e9)} ns")
    return {name: res[k] for k, name in enumerate(out_names)}


def kernel(customer_id, candidate_good, candidate_class, history_goods,
           history_classes, user_table, item_table, cat_table,
           aw1, ab1, aw2, ab2, mw1, mb1, mw2, mb2, mw3, mb3):
    f16 = np.float16
    cid = np.asarray(customer_id).astype(np.int64)
    cg = np.asarray(candidate_good).astype(np.int64)
    cc = np.asarray(candidate_class).astype(np.int64)
    hg = np.asarray(history_goods).astype(np.int64)
    hc = np.asarray(history_classes).astype(np.int64)
    ut = np.asarray(user_table, np.float32)
    it = np.asarray(item_table, np.float32)
    ct = np.asarray(cat_table, np.float32)
    aw1 = np.asarray(aw1, np.float32)
    aw2_ = np.asarray(aw2, np.float32).reshape(80, 1)
    A1, A2, A3, A4 = aw1[0:64], aw1[64:128], aw1[128:192], aw1[192:256]
    mw1 = np.asarray(mw1, np.float32)
    mb1v = np.asarray(mb1, np.float32)
    mw2 = np.asarray(mw2, np.float32)
    mw3 = np.asarray(mw3, np.float32)

    nc = _build_program()
    nc.finalize()

    # ---- host-side gather into compact device layouts (f16) ----
    ieT = it[hg].transpose(2, 0, 1).astype(f16)      # [32, B, 50]
    ceT = ct[hc].transpose(2, 0, 1).astype(f16)      # [32, B, 50]
    maT = np.where(hg == 0, np.float32(MASKV),
                   np.float32(0.0)).astype(f16)[None]  # [1, B, 50]
    HTg = np.concatenate([ieT, ceT, maT], axis=0)    # [65, B, 50]
    HTg = np.concatenate(
        [HTg[:, c * BC:(c + 1) * BC] for c in range(NCORES)], axis=0)
    CAg = np.concatenate([it[cg].T, ct[cc].T], axis=0).astype(f16)  # [64, B]
    CAg = np.concatenate(
        [CAg[:, c * BC:(c + 1) * BC] for c in range(NCORES)], axis=0)
    UTg = ut[cid].T.astype(f16)                       # [32, B]
    UTg = np.concatenate(
        [UTg[:, c * BC:(c + 1) * BC] for c in range(NCORES)], axis=0)

    W1H = np.ascontiguousarray(A2 - A3).astype(f16)
    W1P = np.ascontiguousarray(A4).astype(f16)
    WQC = np.ascontiguousarray(A1 + A3).astype(f16)
    AB1 = np.asarray(ab1, np.float32).reshape(80, 1)
    MB1w = np.stack([mb1v[0:128], mb1v[128:256]], axis=1)

    def rep(a):
        return np.concatenate([a] * NCORES, axis=0)

    global_ins = dict(
        HT=HTg, CA=CAg, UT=UTg,
        W1H=rep(W1H), W1P=rep(W1P), WQC=rep(WQC), AB1=rep(AB1),
        A2E=rep(aw2_.astype(f16)),
        M1U=rep(mw1[0:32].astype(f16)), M1C=rep(mw1[32:96].astype(f16)),
        M1A=rep(mw1[96:160].astype(f16)), MB1=rep(MB1w),
        M2A=rep(mw2[0:128].astype(f16)), M2B=rep(mw2[128:256].astype(f16)),
        MB2=rep(np.asarray(mb2, np.float32).reshape(128, 1)),
        M3=rep(mw3.astype(f16)),
        MB3=rep(np.asarray(mb3, np.float32).reshape(1, 1)),
    )
    res = _run(nc, global_ins, NCORES)
    return res["out"].reshape(-1).astype(np.float32)


# revision 11
# speedup vs baseline: 5.1546x; 1.4968x over previous
import time

import numpy as np
import ml_dtypes

import concourse.bass as bass
import concourse.mybir as mybir
from concourse.bacc import Bacc
from concourse.tile import TileContext

F8 = mybir.dt.float8e4
F16 = mybir.dt.float16
F32 = mybir.dt.float32

B, L, D = 16384, 50, 32
NCORES = 8
BC = B // NCORES            # 2048 samples per core
SCH = 8                     # samples per loop chunk
TCH = SCH * L               # 400 tokens per chunk
MASKV = -240.0              # max-magnitude finite f8e4m3 value
NW = 1266                   # packed f16 weight columns


def _build_program():
    nc = Bacc()
    f8, f16, f32 = F8, F16, F32
    AF = mybir.ActivationFunctionType
    ds = bass.ds

    HT = nc.dram_tensor("HT", [65, BC, L], f8, kind="ExternalInput")
    CAUT = nc.dram_tensor("CAUT", [96, BC], f16, kind="ExternalInput")
    WPH = nc.dram_tensor("WPH", [128, NW], f16, kind="ExternalInput")
    WPF = nc.dram_tensor("WPF", [128, 5], f32, kind="ExternalInput")
    OUT = nc.dram_tensor("out", [1, BC], f32, kind="ExternalOutput")

    with TileContext(nc) as tc:
        with (
            tc.tile_pool(name="const", bufs=1) as cp,
            tc.tile_pool(name="ht", bufs=2) as htp,
            tc.tile_pool(name="work", bufs=2) as wp,
            tc.tile_pool(name="psA", bufs=1, space="PSUM") as psA,
            tc.tile_pool(name="psS", bufs=1, space="PSUM") as psS,
            tc.tile_pool(name="psE", bufs=1, space="PSUM") as psE,
            tc.tile_pool(name="psM", bufs=2, space="PSUM") as psM,
        ):
            # ---- constants from packed tensors ----
            w1h = cp.tile([64, 80], f16, tag="w1h")
            nc.sync.dma_start(out=w1h[:, :], in_=WPH[0:64, 0:80])
            w1p = cp.tile([64, 80], f16, tag="w1p")
            nc.sync.dma_start(out=w1p[:, :], in_=WPH[0:64, 80:160])
            wqc = cp.tile([64, 80], f16, tag="wqc")
            nc.sync.dma_start(out=wqc[:, :], in_=WPH[0:64, 160:240])
            a2e = cp.tile([80, 1], f16, tag="a2e")
            nc.sync.dma_start(out=a2e[:, :], in_=WPH[0:80, 240:241])
            m1u = cp.tile([32, 256], f16, tag="m1u")
            nc.sync.dma_start(out=m1u[:, :], in_=WPH[0:32, 241:497])
            m1c = cp.tile([64, 256], f16, tag="m1c")
            nc.sync.dma_start(out=m1c[:, :], in_=WPH[0:64, 497:753])
            m1a = cp.tile([64, 256], f16, tag="m1a")
            nc.sync.dma_start(out=m1a[:, :], in_=WPH[0:64, 753:1009])
            m2a = cp.tile([128, 128], f16, tag="m2a")
            nc.sync.dma_start(out=m2a[:, :], in_=WPH[:, 1009:1137])
            m2b = cp.tile([128, 128], f16, tag="m2b")
            nc.sync.dma_start(out=m2b[:, :], in_=WPH[:, 1137:1265])
            m3 = cp.tile([128, 1], f16, tag="m3")
            nc.sync.dma_start(out=m3[:, :], in_=WPH[:, 1265:1266])
            ab1 = cp.tile([80, 1], f32, tag="ab1")
            nc.sync.dma_start(out=ab1[:, :], in_=WPF[0:80, 0:1])
            mb1 = cp.tile([128, 2], f32, tag="mb1")
            nc.sync.dma_start(out=mb1[:, :], in_=WPF[:, 1:3])
            mb2 = cp.tile([128, 1], f32, tag="mb2")
            nc.sync.dma_start(out=mb2[:, :], in_=WPF[:, 3:4])
            mb3 = cp.tile([1, 1], f32, tag="mb3")
            nc.sync.dma_start(out=mb3[:, :], in_=WPF[0:1, 4:5])
            ca = cp.tile([64, BC], f16, tag="ca")
            nc.sync.dma_start(out=ca[:, :], in_=CAUT[0:64, :])
            ut = cp.tile([32, BC], f16, tag="ut")
            nc.sync.dma_start(out=ut[:, :], in_=CAUT[64:96, :])
            ones1 = cp.tile([1, 64], f16, tag="ones1")
            nc.vector.memset(ones1[:, :], 1.0)
            onesm = cp.tile([65, 1], f16, tag="onesm")
            nc.vector.memset(onesm[:, :], 1.0)

            attS = cp.tile([64, BC], f32, tag="attS")
            denS = cp.tile([1, BC], f32, tag="denS")
            attn = cp.tile([64, BC], f16, tag="attn")
            rec = cp.tile([1, BC], f32, tag="rec")
            rech = cp.tile([1, BC], f16, tag="rech")
            z1a = cp.tile([128, BC], f16, tag="z1a")
            z1b = cp.tile([128, BC], f16, tag="z1b")
            z2t = cp.tile([128, BC], f16, tag="z2")
            outs = cp.tile([1, BC], f32, tag="outs")

            # ---- fused attention loop: 8 samples (400 tokens) per iter ----
            with tc.For_i(0, BC, SCH) as i:
                ht8 = htp.tile([65, TCH], f8)
                nc.sync.dma_start(
                    out=ht8[:, :].rearrange("p (s l) -> p s l", l=L),
                    in_=HT[:, ds(i, SCH), :])
                ht = wp.tile([65, TCH], f16)
                nc.scalar.activation(ht[:, :], ht8[:, :], AF.Copy)
                prod = wp.tile([64, TCH], f16)
                nc.vector.tensor_tensor(
                    out=prod[:, :].rearrange("p (s l) -> p s l", l=L),
                    in0=ht[0:64, :].rearrange("p (s l) -> p s l", l=L),
                    in1=ca[:, ds(i, SCH)].to_broadcast([64, SCH, L]),
                    op=mybir.AluOpType.mult)
                ps = psA.tile([80, TCH], f32)
                nc.tensor.matmul(ps[:, :], w1h[:, :], ht[0:64, :],
                                 start=True, stop=False)
                nc.tensor.matmul(ps[:, :], w1p[:, :], prod[:, :],
                                 start=False, stop=False)
                nc.tensor.matmul(ps[:, :],
                                 wqc[:, :],
                                 ca[:, ds(i, SCH)].to_broadcast([64, SCH, L]),
                                 start=False, stop=True)
                h80 = wp.tile([80, TCH], f16)
                nc.scalar.activation(h80[:, :], ps[:, :], AF.Relu,
                                     bias=ab1[:, :])
                ss = psS.tile([1, TCH], f32)
                nc.tensor.matmul(ss[:, :], a2e[:, :], h80[:, :],
                                 start=True, stop=False)
                nc.tensor.matmul(ss[:, :], onesm[64:65, :], ht[64:65, :],
                                 start=False, stop=True)
                e = wp.tile([1, TCH], f16)
                nc.scalar.activation(e[:, :], ss[:, :], AF.Exp)
                ebc = psE.tile([64, TCH], f32)
                nc.tensor.matmul(ebc[:, :], ones1[:, :], e[:, :],
                                 start=True, stop=True)
                wh = wp.tile([64, TCH], f32)
                nc.vector.tensor_tensor(out=wh[:, :], in0=ht[0:64, :],
                                        in1=ebc[:, :],
                                        op=mybir.AluOpType.mult)
                nc.vector.tensor_reduce(
                    out=attS[:, ds(i, SCH)],
                    in_=wh[:, :].rearrange("p (s l) -> p s l", l=L),
                    axis=mybir.AxisListType.X, op=mybir.AluOpType.add)
                nc.vector.tensor_reduce(
                    out=denS[:, ds(i, SCH)],
                    in_=e[:, :].rearrange("p (s l) -> p s l", l=L),
                    axis=mybir.AxisListType.X, op=mybir.AluOpType.add)

            # ---- normalize attention ----
            nc.vector.tensor_scalar_add(rec[:, :], denS[:, :], 1e-20)
            nc.vector.reciprocal(rec[:, :], rec[:, :])
            nc.scalar.activation(rech[:, :], rec[:, :], AF.Copy)
            CH = 512
            for q in range(BC // CH):
                off = q * CH
                rb = psM.tile([64, CH], f32, tag="m")
                nc.tensor.matmul(rb[:, :], ones1[:, :], rech[:, off:off + CH],
                                 start=True, stop=True)
                nc.vector.tensor_tensor(out=attn[:, off:off + CH],
                                        in0=attS[:, off:off + CH],
                                        in1=rb[:, :],
                                        op=mybir.AluOpType.mult)

            # ---- final MLP ----
            for q in range(BC // CH):
                off = q * CH
                sl = slice(off, off + CH)
                for mh in range(2):
                    mc = mh * 128
                    zp = psM.tile([128, CH], f32, tag="m")
                    nc.tensor.matmul(zp[:, :], m1u[:, mc:mc + 128],
                                     ut[:, sl], start=True, stop=False)
                    nc.tensor.matmul(zp[:, :], m1c[:, mc:mc + 128],
                                     ca[:, sl], start=False, stop=False)
                    nc.tensor.matmul(zp[:, :], m1a[:, mc:mc + 128],
                                     attn[:, sl], start=False, stop=True)
                    zt = z1a if mh == 0 else z1b
                    nc.scalar.activation(zt[:, sl], zp[:, :], AF.Relu,
                                         bias=mb1[:, mh:mh + 1])
                z2p = psM.tile([128, CH], f32, tag="m")
                nc.tensor.matmul(z2p[:, :], m2a[:, :], z1a[:, sl],
                                 start=True, stop=False)
                nc.tensor.matmul(z2p[:, :], m2b[:, :], z1b[:, sl],
                                 start=False, stop=True)
                nc.scalar.activation(z2t[:, sl], z2p[:, :], AF.Relu,
                                     bias=mb2[:, :])
                z3p = psM.tile([1, CH], f32, tag="m")
                nc.tensor.matmul(z3p[:, :], m3[:, :], z2t[:, sl],
                                 start=True, stop=True)
                nc.scalar.activation(outs[:, off:off + CH], z3p[:, :], AF.Copy)
            nc.vector.tensor_scalar_add(outs[:, :], outs[:, :], mb3[0:1, 0:1])
            nc.sync.dma_start(out=OUT[:, :], in_=outs[:, :])
    return nc


def _run(nc, global_ins, n_cores):
    """Execute the finalized program on n_cores via PJRT (axon).

    Inputs are shipped with per-device device_put (async) so the transfer
    overlaps the NEFF compile/load; the jit call then consumes resident
    arrays. The printed time covers put issue + compile + execute + fetch.
    """
    import jax
    from jax.sharding import Mesh, PartitionSpec, NamedSharding
    try:
        from jax import shard_map
        def _smap(f, mesh, in_specs, out_specs):
            return shard_map(f, mesh=mesh, in_specs=in_specs,
                             out_specs=out_specs, check_vma=False)
    except ImportError:
        from jax.experimental.shard_map import shard_map
        def _smap(f, mesh, in_specs, out_specs):
            return shard_map(f, mesh=mesh, in_specs=in_specs,
                             out_specs=out_specs, check_rep=False)
    from concourse import bass2jax

    devs = jax.devices()[:n_cores]
    bass2jax.install_neuronx_cc_hook()

    partition_name = (nc.partition_id_tensor.name
                      if nc.partition_id_tensor else None)
    in_names, out_names, out_avals, zero_outs = [], [], [], []
    for alloc in nc.m.functions[0].allocations:
        if not isinstance(alloc, mybir.MemoryLocationSet):
            continue
        name = alloc.memorylocations[0].name
        if alloc.kind == "ExternalInput":
            if name != partition_name:
                in_names.append(name)
        elif alloc.kind == "ExternalOutput":
            shape = tuple(alloc.tensor_shape)
            dtype = mybir.dt.np(alloc.dtype)
            out_avals.append(jax.core.ShapedArray(shape, dtype))
            out_names.append(name)
            zero_outs.append(np.zeros((n_cores * shape[0], *shape[1:]), dtype))
    n_params = len(in_names)
    n_outs = len(out_avals)
    all_names = list(in_names) + list(out_names)
    if partition_name is not None:
        all_names.append(partition_name)

    def _body(*args):
        operands = list(args)
        if partition_name is not None:
            operands.append(bass2jax.partition_id_tensor())
        return tuple(bass2jax._bass_exec_p.bind(
            *operands, out_avals=tuple(out_avals), in_names=tuple(all_names),
            out_names=tuple(out_names), lowering_input_output_aliases=(),
            sim_require_finite=True, sim_require_nnan=True, nc=nc))

    donate = tuple(range(n_params, n_params + n_outs))
    mesh = Mesh(np.asarray(devs), ("core",))
    sharded = jax.jit(
        _smap(_body, mesh,
              (PartitionSpec("core"),) * (n_params + n_outs),
              (PartitionSpec("core"),) * n_outs),
        donate_argnums=donate, keep_unused=True)

    t0 = time.time()
    # async per-device puts: transfer streams while the NEFF compiles below
    sh = NamedSharding(mesh, PartitionSpec("core"))
    dev_in = []
    for name in in_names:
        a = global_ins[name]
        per = a.shape[0] // n_cores
        shards = [jax.device_put(a[c * per:(c + 1) * per], devs[c])
                  for c in range(n_cores)]
        dev_in.append(jax.make_array_from_single_device_arrays(
            a.shape, sh, shards))
    dev_zero = []
    for z in zero_outs:
        per = z.shape[0] // n_cores
        shards = [jax.device_put(z[c * per:(c + 1) * per], devs[c])
                  for c in range(n_cores)]
        dev_zero.append(jax.make_array_from_single_device_arrays(
            z.shape, sh, shards))

    tp = time.time()
    compiled = sharded.lower(*dev_in, *dev_zero).compile()
    tc_ = time.time()
    out_arrs = compiled(*dev_in, *dev_zero)
    res = [np.asarray(o) for o in out_arrs]
    t1 = time.time()
    print(f"[breakdown] put-issue {tp - t0:.2f}s  compile {tc_ - tp:.2f}s  "
          f"exec+fetch {t1 - tc_:.2f}s")
    print(f"HW exec time: {int((t1 - t0) * 1e9)} ns")
    return {name: res[k] for k, name in enumerate(out_names)}


def kernel(customer_id, candidate_good, candidate_class, history_goods,
           history_classes, user_table, item_table, cat_table,
           aw1, ab1, aw2, ab2, mw1, mb1, mw2, mb2, mw3, mb3):
    f16 = np.float16
    f8 = ml_dtypes.float8_e4m3
    cid = np.asarray(customer_id).astype(np.int64)
    cg = np.asarray(candidate_good).astype(np.int64)
    cc = np.asarray(candidate_class).astype(np.int64)
    hg = np.asarray(history_goods).astype(np.int64)
    hc = np.asarray(history_classes).astype(np.int64)
    ut = np.asarray(user_table, np.float32)
    it = np.asarray(item_table, np.float32)
    ct = np.asarray(cat_table, np.float32)
    aw1 = np.asarray(aw1, np.float32)
    aw2_ = np.asarray(aw2, np.float32).reshape(80, 1)
    A1, A2, A3, A4 = aw1[0:64], aw1[64:128], aw1[128:192], aw1[192:256]
    mw1 = np.asarray(mw1, np.float32)
    mb1v = np.asarray(mb1, np.float32)
    mw2 = np.asarray(mw2, np.float32)
    mw3 = np.asarray(mw3, np.float32)

    nc = _build_program()
    nc.finalize()

    # ---- host-side gather into compact device layouts ----
    ieT = it[hg].transpose(2, 0, 1)                  # [32, B, 50]
    ceT = ct[hc].transpose(2, 0, 1)                  # [32, B, 50]
    maT = np.where(hg == 0, np.float32(MASKV),
                   np.float32(0.0))[None]            # [1, B, 50]
    HTg = np.concatenate([ieT, ceT, maT], axis=0).astype(f8)   # [65, B, 50]
    HTg = np.concatenate(
        [HTg[:, c * BC:(c + 1) * BC] for c in range(NCORES)], axis=0)
    CAUTg = np.concatenate([it[cg].T, ct[cc].T, ut[cid].T],
                           axis=0).astype(f16)       # [96, B]
    CAUTg = np.concatenate(
        [CAUTg[:, c * BC:(c + 1) * BC] for c in range(NCORES)], axis=0)

    WPH = np.zeros((128, NW), np.float32)
    WPH[0:64, 0:80] = A2 - A3
    WPH[0:64, 80:160] = A4
    WPH[0:64, 160:240] = A1 + A3
    WPH[0:80, 240:241] = aw2_
    WPH[0:32, 241:497] = mw1[0:32]
    WPH[0:64, 497:753] = mw1[32:96]
    WPH[0:64, 753:1009] = mw1[96:160]
    WPH[:, 1009:1137] = mw2[0:128]
    WPH[:, 1137:1265] = mw2[128:256]
    WPH[:, 1265:1266] = mw3
    WPF = np.zeros((128, 5), np.float32)
    WPF[0:80, 0:1] = np.asarray(ab1, np.float32).reshape(80, 1)
    WPF[0:128, 1] = mb1v[0:128]
    WPF[0:128, 2] = mb1v[128:256]
    WPF[:, 3:4] = np.asarray(mb2, np.float32).reshape(128, 1)
    WPF[0, 4] = np.asarray(mb3, np.float32).reshape(())

    def rep(a):
        return np.concatenate([a] * NCORES, axis=0)

    global_ins = dict(
        HT=HTg, CAUT=CAUTg,
        WPH=rep(WPH.astype(f16)), WPF=rep(WPF),
    )
    res = _run(nc, global_ins, NCORES)
    return res["out"].reshape(-1).astype(np.float32)
